# revision 106
# baseline (speedup 1.0000x reference)
"""BCJR decoder (rate-1/2 conv code, 64 states) on 8 Trainium2 cores.

Strategy
--------
Data-parallel over batch: 32 codewords per core. Within a core, each
codeword's T=2048 trellis steps are split into C=16 chunks of 128 steps,
decoded in parallel with L=11 warm-up steps on each side (windowed BCJR).
The time axis is padded with llr_a=+16 "pilot" steps which deterministically
collapse the state to 0, making chunk 0 / chunk 15 boundary conditions exact.
A 4th sign-table row carries a host-set exponent shift that cancels the
pilots' e^{+8}/step growth so normalization can lag without bf16 overflow.

Layout: 128 SBUF partitions = 32 codewords x 4 chunk-groups; 4 more chunks
("f groups") along the free dimension. 150 sequential steps per pass.

Per step: PE fp32r matmul (sign-table x llr quadruple) builds branch-metric
exponents E in PSUM; ScalarE exp(0.5 E) -> G (bf16); VectorE runs the
alpha/beta recursions as two half-width mult + pairsum pairs interleaved so
every semaphore latency hides under a neighboring op. In the forward pass
the second pairsum half runs on the otherwise-idle GPSIMD. Normalization is
folded into E: a DVE reduce + ScalarE ln produce lnZ, and a -2*identity
accumulate matmul subtracts 2 lnZ from a later step's exponents (scale
cancels exactly in the final LLR).

The joint (LLR numerator/denominator) is deferred: the backward pass stores
beta history; GPSIMD streams jm = ahist*bhist plus the first pairwise-sum
stage in 16-column slabs behind the backward recursion, and the DVE finishes
the sum tree, ln, subtract, and output DMA in two pipelined column halves.
"""

import os
from contextlib import ExitStack

import numpy as np

import concourse.bass as bass
import concourse.mybir as mybir
from concourse import tile as tile_mod
from concourse.tile_rust import add_dep_helper
from concourse.bass_utils import run_bass_kernel_spmd

# ---------------- problem constants (hardcoded) ----------------
B_FULL, N_FULL = 256, 4096
T = N_FULL // 2            # 2048 trellis steps
N_CORES = 8
B_CORE = B_FULL // N_CORES  # 32 codewords per core
C = 16                     # time chunks per codeword
S = T // C                 # 128 steps per chunk
L = 11                     # warmup steps each side
TL = S + 2 * L             # 192 local steps
CF = 4                     # chunks in free dim (C = 4 partition-groups * CF)
PAD_A = 16.0               # llr_a pad value (forces state collapse)
NORM_EVERY = 8

F32 = mybir.dt.float32
F32R = mybir.dt.float32r
BF16 = mybir.dt.bfloat16


def _sign_table():
    """[4, 128] rows (la, l0, l1, shift) x cols (b, s):
    E[s,b] = sum_c sign[c,(b,s)] * llr_c.  Row 3 (all ones) carries a
    host-chosen per-step additive exponent shift (pad compensation)."""
    gen = ("1111001", "1011011")
    mu = 6
    g = np.array([[int(c) for c in p] for p in gen])
    opf = np.zeros((64, 2), np.int32)
    for s in range(64):
        rbits = [(s >> (mu - 1 - j)) & 1 for j in range(mu)]
        for b in range(2):
            w = np.array([b] + rbits)
            obits = (g @ w) % 2
            opf[s, b] = obits[0] * 2 + obits[1]
    ops = (1.0 - 2.0 * np.array([[(o >> (1 - j)) & 1 for j in range(2)]
                                 for o in range(4)])).astype(np.float32)
    sa = np.concatenate([np.ones(64), -np.ones(64)])
    s0 = np.concatenate([ops[opf[:, 0], 0], ops[opf[:, 1], 0]])
    s1 = np.concatenate([ops[opf[:, 0], 1], ops[opf[:, 1], 1]])
    return np.stack([sa, s0, s1, np.ones(128)]).astype(np.float32)  # [4, 128]


NCOMP = 4
SIGN_NP = _sign_table()
# block-diag [16, 512]: rows (f*4+c), cols (f', (b,s))
SIGN_BD = np.zeros((4 * NCOMP, 512), np.float32)
for _f in range(4):
    SIGN_BD[_f * NCOMP:_f * NCOMP + NCOMP, _f * 128:(_f + 1) * 128] = SIGN_NP
EYE_M2 = (-2.0 * np.eye(128)).astype(np.float32)

# ---------------- bass program ----------------
_NC_CACHE = {}


def _ap(a, offset_extra, dims):
    """Custom AP over the same tensor as `a` (partition dim kept)."""
    return bass.AP(tensor=a.tensor, offset=a.offset + offset_extra,
                   ap=[list(a.ap[0])] + [list(d) for d in dims])


def build_nc():
    nc = bass.Bass()
    llr_t_d = nc.declare_dram_parameter("llr_t", [16, TL * 128], F32R, isOutput=False)
    sign_d = nc.declare_dram_parameter("sign", [16, 512], F32R, isOutput=False)
    eye_d = nc.declare_dram_parameter("eye", [128, 128], F32R, isOutput=False)
    out_d = nc.declare_dram_parameter("llr_out", [B_CORE, T], F32, isOutput=True)
    dbg = os.environ.get("KDBG", "0") == "1"
    if dbg:
        dbg_jsum = nc.declare_dram_parameter("dbg_jsum", [128, S * 8], F32, isOutput=True)
        dbg_ahist = nc.declare_dram_parameter("dbg_ahist", [128, 2 * 256], F32, isOutput=True)
        dbg_g = nc.declare_dram_parameter("dbg_g", [128, 512], F32, isOutput=True)

    W = 25                 # llr_t streaming window (steps)
    NW = TL // W           # 6 windows
    mult = mybir.AluOpType.mult
    add = mybir.AluOpType.add

    with tile_mod.TileContext(nc) as tc, ExitStack() as ctx:
        # static ring buffers (pool alloc/release deps would exceed the
        # 1-sync-wait-per-instruction hardware limit)
        def ring(nm, n, shape, dt=F32):
            return [ctx.enter_context(nc.sbuf_tensor(f"{nm}{i}", shape, dt))
                    for i in range(n)]

        e_ps_bufs = [ctx.enter_context(nc.psum_tensor(f"eps{_i}", [128, 512], F32))
                     for _i in range(8)]
        g_bufs = ring("gbuf", 8, [128, 512], BF16)
        ag_bufs = ring("agbuf", 2, [128, 512], BF16)
        aw_bufs = ring("awbuf", 4, [128, 256], BF16)
        nrm_bufs = ring("nrmbuf", 4, [128, CF])
        lnz_bufs = ring("lnzbuf", 4, [128, CF], F32R)
        lt_bufs = ring("ltbuf", 2, [16, W * 128], F32R)

        dve_scr = ctx.enter_context(nc.sbuf_tensor("dvescr", [1, 8], F32))
        act_scr = ctx.enter_context(nc.sbuf_tensor("actscr", [1, 8], F32))
        act_scr2 = ctx.enter_context(nc.sbuf_tensor("actscr2", [1, 16], F32))
        sign_t = ctx.enter_context(nc.sbuf_tensor("sign_sb", [16, 512], F32R))
        sign_sb = sign_t[:]
        eye_t = ctx.enter_context(nc.sbuf_tensor("eye_sb", [128, 128], F32R))

        state = {"prev_g": None, "dve_stable": None, "lnz": None}
        ahist_t = ctx.enter_context(nc.sbuf_tensor("ahist", [128, S * 256], BF16))
        ahist = ahist_t[:]
        bhist_t = ctx.enter_context(nc.sbuf_tensor("bhist", [128, S * 256], BF16))
        bhist = bhist_t[:]
        jsum_t = ctx.enter_context(nc.sbuf_tensor("jsum", [128, S * 8], F32))
        jsum = jsum_t[:]
        _counters = {"g": 0, "ag": 0, "aw": 0, "nrm": 0, "lnz": 0, "lt": 0}

        def nxt(nm, bufs):
            i = _counters[nm]
            _counters[nm] = i + 1
            return bufs[i % len(bufs)]

        def make_G(tau, lt_sb, fresh_dma, deint=False, apply_lnz=False):
            """PE: E[row,(b,s)] per f into PSUM; ACT: G = exp(0.5 E).

            PE Matmult (LW struct) supports only ONE sync wait, so 1-element
            dummy matmuls absorb the PSUM-WAR and window-DMA waits first.
            When apply_lnz, a second accumulate matmul adds -2*lnZ[p,f] to
            every column of f's block (normalization folded into E).
            """
            e_ps = e_ps_bufs[tau % 8]
            col = (tau % W) * 128
            nc.tensor.matmul(out=e_ps[0:1, 0:1], lhsT=sign_t[0:1, 0:1].bitcast(F32),
                             rhs=sign_t[0:1, 0:1].bitcast(F32), start=True, stop=True)
            if fresh_dma:
                nc.tensor.matmul(out=e_ps[0:1, 0:1],
                                 lhsT=lt_sb[0:1, col:col + 1].bitcast(F32),
                                 rhs=sign_t[0:1, 0:1].bitcast(F32), start=True, stop=True)
            lnz = state["lnz"] if apply_lnz else None
            nc.tensor.matmul(
                out=e_ps[:], lhsT=lt_sb[:, col:col + 128],
                rhs=sign_sb, start=True, stop=lnz is None)
            if lnz is not None:
                lnz_b = _ap(lnz, 0, [[1, CF], [0, 2], [0, 64]])
                nc.tensor.matmul(out=e_ps[:], lhsT=eye_t[:], rhs=lnz_b,
                                 start=False, stop=True)
                state["lnz"] = None
            g_sb = nxt("g", g_bufs)[:]
            q = tau % 8
            # ACT absorber (HW: one sync wait per instruction):
            #   c2a:    DVE wait covering the g-slot WAR (reads a stable
            #           DVE output newer than the old g reader); its hist
            #           join also implies the exp's self-WAW on the g slot
            #   exp:    carries only the PE wait
            dsrc = state.get("dve_stable")
            c2a_src = dsrc if dsrc is not None else sign_sb
            # bwd iterates tau downward: mirror its slots so they don't
            # collide with the fwd tail's recently-written ones
            q16 = (15 - tau % 16) if deint else tau % 16
            i_c2a = nc.scalar.copy(out=act_scr2[0:1, q16:q16 + 1],
                                   in_=c2a_src[0:1, 0:1])
            if deint:
                g_out = _ap(g_sb, 0, [[128, 4], [64, 2], [1, 32], [32, 2]])
            else:
                g_out = g_sb.rearrange("p (f b s) -> p f b s", f=CF, b=2)
            i_exp = nc.scalar.activation(
                out=g_out,
                in_=e_ps[:].rearrange("p (f b s) -> p f b s", f=CF, b=2),
                func=mybir.ActivationFunctionType.Exp, scale=0.5)
            add_dep_helper(i_exp.ins, i_c2a.ins, False, "act-order")
            state["prev_g"] = g_sb
            return g_sb

        def load_window(w):
            lt_sb = nxt("lt", lt_bufs)
            nc.sync.dma_start(out=lt_sb[:],
                              in_=llr_t_d[:, w * W * 128:(w + 1) * W * 128])
            return lt_sb

        def emit_zred(cur):
            """Off-chain normalization: Z = partial state-sum of alpha',
            lnz = ln Z (ACT, f32r); applied later by make_G's -2I accumulate
            matmul. Any positive per-chunk scale cancels exactly in the LLR,
            so summing states 0..15 suffices (state 0 carries the mass in
            pilot-collapsed regimes, and a random quarter tracks magnitude
            otherwise) -- a quarter-width 1x reduce."""
            asum = nxt("nrm", nrm_bufs)[:]
            nc.vector.tensor_reduce(
                out=asum, in_=_ap(cur, 0, [[64, CF], [1, 16]]),
                axis=mybir.AxisListType.X, op=add)
            lnz = nxt("lnz", lnz_bufs)[:]
            nc.scalar.activation(out=lnz, in_=asum,
                                 func=mybir.ActivationFunctionType.Ln)
            state["lnz"] = lnz

        # ---------------- forward ----------------
        # Each step is emitted as [multA, multB, pairsumA, pairsumB] over the
        # f01 / f23 halves: every sem wait lands under the neighboring op, so
        # the recursion runs gap-free on the DVE.
        alpha = nxt("aw", aw_bufs)[:]
        nc.vector.memset(alpha, 1.0 / 64)
        # window 0 first (longest transfer on the serialized HWDGE), then the
        # small sign table, then the non-urgent eye matrix
        # first-step inputs stream first; eye DMA + its absorber are deferred
        # into the loop (eye is first needed at the tau=12 lnZ apply)
        lt_sb = load_window(0)
        nc.sync.dma_start(out=sign_sb, in_=sign_d[:])
        zpend = None
        for tau in range(TL):
            fresh = tau % W == 0
            if fresh and tau > 0:
                lt_sb = load_window(tau // W)
            # stable marker: dvescr slot written by the absorber two steps ago
            # (slot rewritten every 8 steps -> covers the 8-old g-buffer WAR
            # without pulling exp into the recursion chain)
            state["dve_stable"] = (dve_scr[0:1, (tau - 2) % 8:(tau - 2) % 8 + 1]
                                   if tau >= 2 else alpha)
            if tau == 2:
                nc.sync.dma_start(out=eye_t[:], in_=eye_d[:])
            if tau == 3:
                # one-time eye absorber; writes a corner of the PSUM buf that
                # make_G(3)'s start=True matmul immediately resets. PE runs
                # ~2 steps ahead of the recursion, so the eye transfer is
                # done before this slot executes.
                nc.tensor.matmul(out=e_ps_bufs[3][0:1, 0:1],
                                 lhsT=eye_t[0:1, 0:1].bitcast(F32),
                                 rhs=eye_t[0:1, 0:1].bitcast(F32),
                                 start=True, stop=True)
            g_sb = make_G(tau, lt_sb, fresh,
                          apply_lnz=tau % NORM_EVERY == 4 and state["lnz"] is not None)
            ag = nxt("ag", ag_bufs)[:]
            # absorber: carries the A-half alpha wait (so multA carries only
            # the ACT wait; multB carries the B-half alpha wait itself)
            i_d1 = nc.vector.tensor_copy(
                out=dve_scr[0:1, tau % 8:tau % 8 + 1], in_=alpha[0:1, 0:1])
            i_mA = nc.vector.tensor_tensor(
                out=_ap(ag, 0, [[128, 2], [64, 2], [1, 64]]),
                in0=_ap(g_sb, 0, [[128, 2], [64, 2], [1, 64]]),
                in1=_ap(alpha, 0, [[64, 2], [0, 2], [1, 64]]), op=mult)
            add_dep_helper(i_mA.ins, i_d1.ins, False, "dve-order")
            nc.vector.tensor_tensor(
                out=_ap(ag, 256, [[128, 2], [64, 2], [1, 64]]),
                in0=_ap(g_sb, 256, [[128, 2], [64, 2], [1, 64]]),
                in1=_ap(alpha, 128, [[64, 2], [0, 2], [1, 64]]), op=mult)
            # pairsum -> alpha' (prenorm); store to ahist when in output range
            # A-half on DVE; B-half on the (forward-idle) GPSIMD — its result
            # lands just in time for the next step's B-mult
            if L <= tau < L + S:
                dst = ahist[:, (tau - L) * 256:(tau - L + 1) * 256]
            else:
                dst = nxt("aw", aw_bufs)[:]
            nc.vector.tensor_tensor(
                out=_ap(dst, 0, [[64, 2], [32, 2], [1, 32]]),
                in0=_ap(ag, 0, [[128, 2], [64, 2], [2, 32]]),
                in1=_ap(ag, 1, [[128, 2], [64, 2], [2, 32]]), op=add)
            nc.gpsimd.tensor_tensor(
                out=_ap(dst, 128, [[64, 2], [32, 2], [1, 32]]),
                in0=_ap(ag, 256, [[128, 2], [64, 2], [2, 32]]),
                in1=_ap(ag, 257, [[128, 2], [64, 2], [2, 32]]), op=add)
            if zpend is not None:
                # reads a step-old buffer: executes immediately after this
                # step's pairsum, filling the recursion's sem-latency gap
                emit_zred(zpend)
                zpend = None
            alpha = dst
            if tau % NORM_EVERY == NORM_EVERY - 1 and tau <= TL - 4:
                zpend = alpha

        # ---------------- backward ----------------
        # boundary absorber: an explicit DVE wait on the forward pass's last
        # Pool pairsum so later WAW/WAR deps against Pool writes are implied
        nc.vector.tensor_copy(out=dve_scr[0:1, 7:8], in_=alpha[0:1, 128:129])
        beta = nxt("aw", aw_bufs)[:]
        nc.vector.memset(beta, 1.0 / 64)
        zpend = None
        for tau in range(TL - 1, -1, -1):
            # the bwd's first window is the fwd's last -- still resident in
            # SBUF, skip the reload (and its +900ns completion sem at the
            # pass boundary)
            fresh = tau % W == W - 1 and tau != TL - 1
            if fresh:
                lt_sb = load_window(tau // W)
            state["dve_stable"] = (dve_scr[0:1, (tau + 2) % 8:(tau + 2) % 8 + 1]
                                   if tau <= TL - 3 else beta)
            g_sb = make_G(tau, lt_sb, fresh, deint=True,
                          apply_lnz=tau % NORM_EVERY == 3 and state["lnz"] is not None)
            bg = nxt("ag", ag_bufs)[:]
            i_d1 = nc.vector.tensor_copy(
                out=dve_scr[0:1, tau % 8:tau % 8 + 1], in_=beta[0:1, 0:1])
            # layout (f, b, m, k): all operands unit-stride innermost (2x mode)
            i_mA = nc.vector.tensor_tensor(
                out=_ap(bg, 0, [[128, 2], [64, 2], [32, 2], [1, 32]]),
                in0=_ap(g_sb, 0, [[128, 2], [64, 2], [32, 2], [1, 32]]),
                in1=_ap(beta, 0, [[64, 2], [32, 2], [0, 2], [1, 32]]), op=mult)
            add_dep_helper(i_mA.ins, i_d1.ins, False, "dve-order")
            nc.vector.tensor_tensor(
                out=_ap(bg, 256, [[128, 2], [64, 2], [32, 2], [1, 32]]),
                in0=_ap(g_sb, 256, [[128, 2], [64, 2], [32, 2], [1, 32]]),
                in1=_ap(beta, 128, [[64, 2], [32, 2], [0, 2], [1, 32]]), op=mult)
            # pairsum -> beta; the beta consumed by step tau-1's joint is this
            # step's output: store it as bhist column tau-1-L for the
            # deferred joint computation
            if L + 1 <= tau <= L + S:
                dst = bhist[:, (tau - 1 - L) * 256:(tau - L) * 256]
            else:
                dst = nxt("aw", aw_bufs)[:]
            # iterate (f, m, k): out idx 64f + 2k + m ; bg idx 128f + b64 + 32m + k
            nc.vector.tensor_tensor(
                out=_ap(dst, 0, [[64, 2], [1, 2], [2, 32]]),
                in0=_ap(bg, 0, [[128, 2], [32, 2], [1, 32]]),
                in1=_ap(bg, 64, [[128, 2], [32, 2], [1, 32]]), op=add)
            # B-half: the first 10 bwd steps write only aw scratch (no bhist)
            # and run before any jm slab exists -> the Pool is idle there and
            # can take the pairsum exactly like the forward pass
            pb_eng = nc.gpsimd if tau >= TL - 10 else nc.vector
            pb_eng.tensor_tensor(
                out=_ap(dst, 128, [[64, 2], [1, 2], [2, 32]]),
                in0=_ap(bg, 256, [[128, 2], [32, 2], [1, 32]]),
                in1=_ap(bg, 320, [[128, 2], [32, 2], [1, 32]]), op=add)
            if zpend is not None:
                emit_zred(zpend)
                zpend = None
            beta = dst
            if tau % NORM_EVERY == 0 and tau >= 3:
                zpend = beta
            # stream the joint computation on the (otherwise idle) GPSIMD:
            # bhist columns > tau-1-L are complete AND no longer live (the
            # current beta is column tau-1-L) -> Pool does jm = ahist*bhist
            # and the first tree stage for a 16-column slab, overlapping the
            # recursion
            k2 = tau - L
            if 0 <= k2 < S and k2 % 16 == 0:
                base = k2 * 256
                sl = slice(k2 * 256, (k2 + 16) * 256)
                nc.gpsimd.tensor_tensor(out=bhist[:, sl], in0=bhist[:, sl],
                                        in1=ahist[:, sl], op=mult)
                dims = [[256, 16], [32, 8], [1, 16]]
                nc.gpsimd.tensor_tensor(
                    out=_ap(bhist, base, dims), in0=_ap(bhist, base, dims),
                    in1=_ap(bhist, base + 16, dims), op=add)

        # ---------------- deferred joint: remaining tree stages (DVE) -----
        # two column-halves so the Ln/sub/DMA of half 0 overlap half 1's tree
        H = S // 2
        lg_t = ctx.enter_context(nc.sbuf_tensor("lgbuf", [128, S * 8], F32))
        lg = lg_t[:]
        llr_t2 = ctx.enter_context(nc.sbuf_tensor("llrsb", [128, 512], F32))
        llr_sb = llr_t2

        def tree_half(h, prev_fin=None):
            cb, jb = h * H * 256, h * H * 8
            first = None
            for w in (8, 4, 2):
                dims = [[256, H], [32, 8], [1, w]]
                i_t = nc.vector.tensor_tensor(
                    out=_ap(bhist, cb, dims), in0=_ap(bhist, cb, dims),
                    in1=_ap(bhist, cb + w, dims), op=add)
                if first is None:
                    first = i_t
            i_fin = nc.vector.tensor_tensor(
                out=_ap(jsum, jb, [[8, H], [1, 8], [1, 1]]),
                in0=_ap(bhist, cb, [[256, H], [32, 8], [1, 1]]),
                in1=_ap(bhist, cb + 1, [[256, H], [32, 8], [1, 1]]), op=add)
            nc.scalar.activation(out=lg[:, jb:jb + H * 8],
                                 in_=jsum[:, jb:jb + H * 8],
                                 func=mybir.ActivationFunctionType.Ln)
            if prev_fin is not None:
                # keep the scheduler from interleaving the halves stage-major
                # (half 0's Ln must overlap half 1's tree)
                add_dep_helper(first.ins, prev_fin.ins, False, "half-order")
            return i_fin

        def sub_half(h):
            jb = h * H * 8
            nc.vector.tensor_tensor(
                out=_ap(llr_sb[:], h * H, [[128, CF], [1, H]]),
                in0=_ap(lg, jb, [[2, CF], [8, H]]),
                in1=_ap(lg, jb + 1, [[2, CF], [8, H]]),
                op=mybir.AluOpType.subtract)

        # half 1 first: its Pool jm-slabs (high columns) finish early in the
        # backward pass, while half 0's last slab lands only at the very end
        def out_dma(h):
            src = _ap(llr_sb[:], h * H, [[128, CF], [1, H]])
            dstp = bass.AP(tensor=out_d[:].tensor, offset=h * H,
                           ap=[[2048, 32], [512, 4], [128, 4], [1, H]])
            nc.sync.dma_start(out=dstp, in_=src)

        fin1 = tree_half(1)
        tree_half(0, prev_fin=fin1)
        sub_half(1)
        sub_half(0)
        # single output DMA: split halves land on different HW queues (8-way
        # round-robin) and the final drain can carry only one wait
        src_ap = llr_sb[:].rearrange("p (f k) -> p f k", f=4)
        dst_ap = bass.AP(tensor=out_d[:].tensor, offset=0,
                         ap=[[2048, 32], [512, 4], [128, 4], [1, 128]])
        nc.sync.dma_start(out=dst_ap, in_=src_ap)

        if dbg:
            nc.sync.dma_start(out=dbg_jsum[:], in_=jsum)
            nc.sync.dma_start(out=dbg_ahist[:, 0:256], in_=ahist[:, 0:256])
            nc.sync.dma_start(out=dbg_ahist[:, 256:512], in_=ahist[:, 127 * 256:128 * 256])
            nc.sync.dma_start(out=dbg_g[:], in_=g_bufs[0][:])
    return nc


_ENG_SELF = {"PE": "PE_", "DVE": "DVE_", "Activation": "Activation_",
             "Pool": "Pool_", "SP": "SP_"}


def _prune_waits(nc):
    """Drop sem waits already implied, so each instruction carries <=1.

    HW structs accept one sync wait per instruction. Tile emits waits that
    are provably satisfied at issue. Vector-clock rules:
      - cross-engine sems: knowledge from transitive joins of kept waits
      - self sems (same engine): only monotone vs explicitly-waited values
        (ACT/DVE completion is not implied by issue order); PE and DMA
        queues complete in order, so own-increment knowledge counts there.
    """
    know = {}        # proc -> {sem_id: known completed value}
    safe = {}        # proc -> {sem_id: completion-proven value (waits/joins
                     #          only, no own-increment issue-order knowledge)}
    waited_max = {}  # proc -> {sem_id: max explicitly waited}
    sem_total = {}   # sem_id -> running total
    hist = {}        # sem_id -> [(total_after, snapshot)]
    out_dma_sems = set()
    bad = []
    for b in nc.m.functions[0].blocks:
        for i in b.instructions:
            si = i.sync_info
            op = str(getattr(i, "opcode", type(i).__name__))
            if si is None:
                continue
            upds = [u for u in (si.on_update or [])
                    if u.sync_type == "semaphore"
                    and u.update_mode in ("sem-inc", "sem-add-imm")]
            if "DMACopy" in op and upds:
                proc = str(upds[0].ant_name)
                outs = getattr(i, "outs", None) or []
                if outs and "llr_out" in str(getattr(outs[0], "memref", "")):
                    out_dma_sems.add(upds[0].id)
            else:
                proc = getattr(i.engine, "value", str(i.engine))
            k = know.setdefault(proc, {})
            ks = safe.setdefault(proc, {})
            wm = waited_max.setdefault(proc, {})
            in_order = proc == "PE" or proc.startswith("DMAHW")
            if "Drain" in op and si.on_wait and len(si.on_wait) > 1:
                best = {}
                for w in si.on_wait:
                    if w.id in out_dma_sems and (
                            w.id not in best
                            or (w.wait_value or 0) > (best[w.id].wait_value or 0)):
                        best[w.id] = w
                si.on_wait = list(best.values())
                continue
            skip = ("Drain" in op) or ("EventSem" in op)
            ow = list(si.on_wait or [])
            if ow and not skip:
                # evaluate cross-engine waits first: their hist joins can
                # prove completion of this engine's own earlier instructions,
                # letting the self-wait checks below prune
                def _selfish(w):
                    nm = str(w.ant_name)
                    return nm == proc or nm.startswith(proc + "_")
                ow.sort(key=_selfish)
                keep = []
                for w in ow:
                    if (w.sync_type != "semaphore"
                            or w.wait_mode != "sem-ge-imm"
                            or w.wait_value is None
                            or str(w.ant_name).startswith("barrier")):
                        keep.append(w)
                        continue
                    v = w.wait_value
                    nm = str(w.ant_name)
                    is_self = nm == proc or nm.startswith(proc + "_")
                    if is_self:
                        implied = (wm.get(w.id, -1) >= v
                                   or ks.get(w.id, 0) >= v
                                   or (in_order and k.get(w.id, 0) >= v))
                    else:
                        implied = (k.get(w.id, 0) >= v
                                   or wm.get(w.id, -1) >= v)
                    if implied:
                        continue
                    keep.append(w)
                    wm[w.id] = max(wm.get(w.id, -1), v)
                    for tot, snap in hist.get(w.id, ()):
                        if tot >= v:
                            for s2, v2 in snap.items():
                                if k.get(s2, 0) < v2:
                                    k[s2] = v2
                                if ks.get(s2, 0) < v2:
                                    ks[s2] = v2
                            break
                    if k.get(w.id, 0) < v:
                        k[w.id] = v
                    if ks.get(w.id, 0) < v:
                        ks[w.id] = v
                if len(keep) != len(ow):
                    si.on_wait = keep
                    ow = keep
                if len(ow) > 1:
                    bad.append((i.name, op,
                                [(x.ant_name, x.wait_value) for x in ow]))
            for u in upds:
                tot = sem_total.get(u.id, 0) + (u.update_value or 0)
                sem_total[u.id] = tot
                k[u.id] = tot
                hist.setdefault(u.id, []).append((tot, dict(k)))
    if bad:
        raise RuntimeError(f"{len(bad)} insts still multi-wait: {bad[:8]}")
    return nc


def _get_nc():
    if "nc" not in _NC_CACHE:
        _NC_CACHE["nc"] = _prune_waits(build_nc())
    return _NC_CACHE["nc"]


# ---------------- host-side layout ----------------
def _prep_core(llr_ch_c, llr_a_c):
    """llr_ch_c [32, 4096], llr_a_c [32, 2048] -> llr_t [16, TL*128] f32."""
    lc = np.zeros((B_CORE, T + 2 * L, 2), np.float32)
    lc[:, L:L + T] = llr_ch_c.reshape(B_CORE, T, 2)
    la = np.full((B_CORE, T + 2 * L), PAD_A, np.float32)
    la[:, L:L + T] = llr_a_c
    # pad-step exponent shift: cancels the e^{+8}/step growth of the la=+16
    # pilots so normalization can lag a few steps without bf16 overflow
    sh1 = np.zeros(T + 2 * L, np.float32)
    sh1[:L] = -PAD_A
    sh1[T + L:] = -PAD_A
    sh = np.broadcast_to(sh1, (B_CORE, T + 2 * L))
    # windows [B, C, TL, comp]
    idx = (np.arange(C)[:, None] * S + np.arange(TL)[None, :])  # [C, TL]
    w = np.stack([la[:, idx], lc[:, idx, 0], lc[:, idx, 1], sh[:, idx]], -1)
    # chunk c = g*4+f ; row = cw*4+g ; llr_t[f*4+comp, tau*128+row]
    w = w.reshape(B_CORE, 4, 4, TL, NCOMP)        # [cw, g, f, tau, comp]
    w = w.transpose(2, 4, 3, 0, 1)                # [f, comp, tau, cw, g]
    return np.ascontiguousarray(w.reshape(4 * NCOMP, TL * 128))


def _run(llr_ch, llr_a, trace=False):
    nc = _get_nc()
    in_maps = []
    for core in range(N_CORES):
        sl = slice(core * B_CORE, (core + 1) * B_CORE)
        in_maps.append({
            "llr_t": _prep_core(np.asarray(llr_ch[sl], np.float32),
                                np.asarray(llr_a[sl], np.float32)),
            "sign": SIGN_BD,
            "eye": EYE_M2,
        })
    res = run_bass_kernel_spmd(nc, in_maps, core_ids=list(range(N_CORES)),
                               trace=trace)
    out = np.concatenate([r["llr_out"] for r in res.results], 0)
    return out.astype(np.float32), res


def kernel(llr_ch, llr_a):
    out, _ = _run(llr_ch, llr_a, trace=False)
    return out



# revision 109
# speedup vs baseline: 1.0016x; 1.0016x over previous
"""BCJR decoder (rate-1/2 conv code, 64 states) on 8 Trainium2 cores.

Strategy
--------
Data-parallel over batch: 32 codewords per core. Within a core, each
codeword's T=2048 trellis steps are split into C=16 chunks of 128 steps,
decoded in parallel with L=11 warm-up steps on each side (windowed BCJR).
The time axis is padded with llr_a=+16 "pilot" steps which deterministically
collapse the state to 0, making chunk 0 / chunk 15 boundary conditions exact.
A 4th sign-table row carries a host-set exponent shift that cancels the
pilots' e^{+8}/step growth so normalization can lag without bf16 overflow.

Layout: 128 SBUF partitions = 32 codewords x 4 chunk-groups; 4 more chunks
("f groups") along the free dimension. 150 sequential steps per pass.

Per step: PE fp32r matmul (sign-table x llr quadruple) builds branch-metric
exponents E in PSUM; ScalarE exp(0.5 E) -> G (bf16); VectorE runs the
alpha/beta recursions as two half-width mult + pairsum pairs interleaved so
every semaphore latency hides under a neighboring op. In the forward pass
the second pairsum half runs on the otherwise-idle GPSIMD. Normalization is
folded into E: a DVE reduce + ScalarE ln produce lnZ, and a -2*identity
accumulate matmul subtracts 2 lnZ from a later step's exponents (scale
cancels exactly in the final LLR).

The joint (LLR numerator/denominator) is deferred: the backward pass stores
beta history; GPSIMD streams jm = ahist*bhist plus the first pairwise-sum
stage in 16-column slabs behind the backward recursion, and the DVE finishes
the sum tree, ln, subtract, and output DMA in two pipelined column halves.
"""

import os
from contextlib import ExitStack

import numpy as np

import concourse.bass as bass
import concourse.mybir as mybir
from concourse import tile as tile_mod
from concourse.tile_rust import add_dep_helper
from concourse.bass_utils import run_bass_kernel_spmd

# ---------------- problem constants (hardcoded) ----------------
B_FULL, N_FULL = 256, 4096
T = N_FULL // 2            # 2048 trellis steps
N_CORES = 8
B_CORE = B_FULL // N_CORES  # 32 codewords per core
C = 16                     # time chunks per codeword
S = T // C                 # 128 steps per chunk
L = 11                     # warmup steps each side
TL = S + 2 * L             # 192 local steps
CF = 4                     # chunks in free dim (C = 4 partition-groups * CF)
PAD_A = 16.0               # llr_a pad value (forces state collapse)
NORM_EVERY = 8

F32 = mybir.dt.float32
F32R = mybir.dt.float32r
BF16 = mybir.dt.bfloat16


def _sign_table():
    """[4, 128] rows (la, l0, l1, shift) x cols (b, s):
    E[s,b] = sum_c sign[c,(b,s)] * llr_c.  Row 3 (all ones) carries a
    host-chosen per-step additive exponent shift (pad compensation)."""
    gen = ("1111001", "1011011")
    mu = 6
    g = np.array([[int(c) for c in p] for p in gen])
    opf = np.zeros((64, 2), np.int32)
    for s in range(64):
        rbits = [(s >> (mu - 1 - j)) & 1 for j in range(mu)]
        for b in range(2):
            w = np.array([b] + rbits)
            obits = (g @ w) % 2
            opf[s, b] = obits[0] * 2 + obits[1]
    ops = (1.0 - 2.0 * np.array([[(o >> (1 - j)) & 1 for j in range(2)]
                                 for o in range(4)])).astype(np.float32)
    sa = np.concatenate([np.ones(64), -np.ones(64)])
    s0 = np.concatenate([ops[opf[:, 0], 0], ops[opf[:, 1], 0]])
    s1 = np.concatenate([ops[opf[:, 0], 1], ops[opf[:, 1], 1]])
    return np.stack([sa, s0, s1, np.ones(128)]).astype(np.float32)  # [4, 128]


NCOMP = 4
SIGN_NP = _sign_table()
# block-diag [16, 512]: rows (f*4+c), cols (f', (b,s))
SIGN_BD = np.zeros((4 * NCOMP, 512), np.float32)
for _f in range(4):
    SIGN_BD[_f * NCOMP:_f * NCOMP + NCOMP, _f * 128:(_f + 1) * 128] = SIGN_NP
EYE_M2 = (-2.0 * np.eye(128)).astype(np.float32)

# ---------------- bass program ----------------
_NC_CACHE = {}


def _ap(a, offset_extra, dims):
    """Custom AP over the same tensor as `a` (partition dim kept)."""
    return bass.AP(tensor=a.tensor, offset=a.offset + offset_extra,
                   ap=[list(a.ap[0])] + [list(d) for d in dims])


def build_nc():
    nc = bass.Bass()
    llr_t_d = nc.declare_dram_parameter("llr_t", [16, TL * 128], F32R, isOutput=False)
    sign_d = nc.declare_dram_parameter("sign", [16, 512], F32R, isOutput=False)
    eye_d = nc.declare_dram_parameter("eye", [128, 128], F32R, isOutput=False)
    out_d = nc.declare_dram_parameter("llr_out", [B_CORE, T], F32, isOutput=True)
    dbg = os.environ.get("KDBG", "0") == "1"
    if dbg:
        dbg_jsum = nc.declare_dram_parameter("dbg_jsum", [128, S * 8], F32, isOutput=True)
        dbg_ahist = nc.declare_dram_parameter("dbg_ahist", [128, 2 * 256], F32, isOutput=True)
        dbg_g = nc.declare_dram_parameter("dbg_g", [128, 512], F32, isOutput=True)

    W = 25                 # llr_t streaming window (steps)
    NW = TL // W           # 6 windows
    mult = mybir.AluOpType.mult
    add = mybir.AluOpType.add

    with tile_mod.TileContext(nc) as tc, ExitStack() as ctx:
        # static ring buffers (pool alloc/release deps would exceed the
        # 1-sync-wait-per-instruction hardware limit)
        def ring(nm, n, shape, dt=F32):
            return [ctx.enter_context(nc.sbuf_tensor(f"{nm}{i}", shape, dt))
                    for i in range(n)]

        e_ps_bufs = [ctx.enter_context(nc.psum_tensor(f"eps{_i}", [128, 512], F32))
                     for _i in range(8)]
        g_bufs = ring("gbuf", 8, [128, 512], BF16)
        ag_bufs = ring("agbuf", 2, [128, 512], BF16)
        aw_bufs = ring("awbuf", 4, [128, 256], BF16)
        nrm_bufs = ring("nrmbuf", 4, [128, CF])
        lnz_bufs = ring("lnzbuf", 4, [128, CF], F32R)
        lt_bufs = ring("ltbuf", 2, [16, W * 128], F32R)

        dve_scr = ctx.enter_context(nc.sbuf_tensor("dvescr", [1, 8], F32))
        act_scr = ctx.enter_context(nc.sbuf_tensor("actscr", [1, 8], F32))
        act_scr2 = ctx.enter_context(nc.sbuf_tensor("actscr2", [1, 16], F32))
        sign_t = ctx.enter_context(nc.sbuf_tensor("sign_sb", [16, 512], F32R))
        sign_sb = sign_t[:]
        eye_t = ctx.enter_context(nc.sbuf_tensor("eye_sb", [128, 128], F32R))
        # step-0 window head in its own buffer: DMA-write deps are tracked
        # per-buffer, so the first matmul waits only this tiny transfer
        # instead of the full first window
        lt_head = ctx.enter_context(nc.sbuf_tensor("lthead", [16, 128], F32R))

        state = {"prev_g": None, "dve_stable": None, "lnz": None}
        ahist_t = ctx.enter_context(nc.sbuf_tensor("ahist", [128, S * 256], BF16))
        ahist = ahist_t[:]
        bhist_t = ctx.enter_context(nc.sbuf_tensor("bhist", [128, S * 256], BF16))
        bhist = bhist_t[:]
        jsum_t = ctx.enter_context(nc.sbuf_tensor("jsum", [128, S * 8], F32))
        jsum = jsum_t[:]
        _counters = {"g": 0, "ag": 0, "aw": 0, "nrm": 0, "lnz": 0, "lt": 0}

        def nxt(nm, bufs):
            i = _counters[nm]
            _counters[nm] = i + 1
            return bufs[i % len(bufs)]

        def make_G(tau, lt_sb, fresh_dma, deint=False, apply_lnz=False):
            """PE: E[row,(b,s)] per f into PSUM; ACT: G = exp(0.5 E).

            PE Matmult (LW struct) supports only ONE sync wait, so 1-element
            dummy matmuls absorb the PSUM-WAR and window-DMA waits first.
            When apply_lnz, a second accumulate matmul adds -2*lnZ[p,f] to
            every column of f's block (normalization folded into E).
            """
            e_ps = e_ps_bufs[tau % 8]
            col = (tau % W) * 128
            lt_src = lt_head[:] if tau == 0 and not deint else lt_sb
            lcol = 0 if tau == 0 and not deint else col
            nc.tensor.matmul(out=e_ps[0:1, 0:1], lhsT=sign_t[0:1, 0:1].bitcast(F32),
                             rhs=sign_t[0:1, 0:1].bitcast(F32), start=True, stop=True)
            if fresh_dma:
                nc.tensor.matmul(out=e_ps[0:1, 0:1],
                                 lhsT=lt_src[0:1, lcol:lcol + 1].bitcast(F32),
                                 rhs=sign_t[0:1, 0:1].bitcast(F32), start=True, stop=True)
            lnz = state["lnz"] if apply_lnz else None
            nc.tensor.matmul(
                out=e_ps[:], lhsT=lt_src[:, lcol:lcol + 128],
                rhs=sign_sb, start=True, stop=lnz is None)
            if lnz is not None:
                lnz_b = _ap(lnz, 0, [[1, CF], [0, 2], [0, 64]])
                nc.tensor.matmul(out=e_ps[:], lhsT=eye_t[:], rhs=lnz_b,
                                 start=False, stop=True)
                state["lnz"] = None
            g_sb = nxt("g", g_bufs)[:]
            q = tau % 8
            # ACT absorber (HW: one sync wait per instruction):
            #   c2a:    DVE wait covering the g-slot WAR (reads a stable
            #           DVE output newer than the old g reader); its hist
            #           join also implies the exp's self-WAW on the g slot
            #   exp:    carries only the PE wait
            dsrc = state.get("dve_stable")
            c2a_src = dsrc if dsrc is not None else sign_sb
            # bwd iterates tau downward: mirror its slots so they don't
            # collide with the fwd tail's recently-written ones
            q16 = (15 - tau % 16) if deint else tau % 16
            i_c2a = nc.scalar.copy(out=act_scr2[0:1, q16:q16 + 1],
                                   in_=c2a_src[0:1, 0:1])
            if deint:
                g_out = _ap(g_sb, 0, [[128, 4], [64, 2], [1, 32], [32, 2]])
            else:
                g_out = g_sb.rearrange("p (f b s) -> p f b s", f=CF, b=2)
            i_exp = nc.scalar.activation(
                out=g_out,
                in_=e_ps[:].rearrange("p (f b s) -> p f b s", f=CF, b=2),
                func=mybir.ActivationFunctionType.Exp, scale=0.5)
            add_dep_helper(i_exp.ins, i_c2a.ins, False, "act-order")
            state["prev_g"] = g_sb
            return g_sb

        def load_window(w):
            lt_sb = nxt("lt", lt_bufs)
            nc.sync.dma_start(out=lt_sb[:],
                              in_=llr_t_d[:, w * W * 128:(w + 1) * W * 128])
            return lt_sb

        def emit_zred(cur):
            """Off-chain normalization: Z = partial state-sum of alpha',
            lnz = ln Z (ACT, f32r); applied later by make_G's -2I accumulate
            matmul. Any positive per-chunk scale cancels exactly in the LLR,
            so summing states 0..15 suffices (state 0 carries the mass in
            pilot-collapsed regimes, and a random quarter tracks magnitude
            otherwise) -- a quarter-width 1x reduce."""
            asum = nxt("nrm", nrm_bufs)[:]
            nc.vector.tensor_reduce(
                out=asum, in_=_ap(cur, 0, [[64, CF], [1, 16]]),
                axis=mybir.AxisListType.X, op=add)
            lnz = nxt("lnz", lnz_bufs)[:]
            nc.scalar.activation(out=lnz, in_=asum,
                                 func=mybir.ActivationFunctionType.Ln)
            state["lnz"] = lnz

        # ---------------- forward ----------------
        # Each step is emitted as [multA, multB, pairsumA, pairsumB] over the
        # f01 / f23 halves: every sem wait lands under the neighboring op, so
        # the recursion runs gap-free on the DVE.
        alpha = nxt("aw", aw_bufs)[:]
        nc.vector.memset(alpha, 1.0 / 64)
        # window 0 first (longest transfer on the serialized HWDGE), then the
        # small sign table, then the non-urgent eye matrix
        # step-0 critical inputs stream first (sign, 1-step head), then the
        # full first window; eye DMA + absorber are deferred into the loop
        nc.sync.dma_start(out=sign_sb, in_=sign_d[:])
        nc.sync.dma_start(out=lt_head[:], in_=llr_t_d[:, 0:128])
        lt_sb = load_window(0)
        zpend = None
        for tau in range(TL):
            fresh = tau % W == 0
            if fresh and tau > 0:
                lt_sb = load_window(tau // W)
            # stable marker: dvescr slot written by the absorber two steps ago
            # (slot rewritten every 8 steps -> covers the 8-old g-buffer WAR
            # without pulling exp into the recursion chain)
            state["dve_stable"] = (dve_scr[0:1, (tau - 2) % 8:(tau - 2) % 8 + 1]
                                   if tau >= 2 else alpha)
            if tau == 2:
                nc.sync.dma_start(out=eye_t[:], in_=eye_d[:])
            if tau == 3:
                # one-time eye absorber; writes a corner of the PSUM buf that
                # make_G(3)'s start=True matmul immediately resets. PE runs
                # ~2 steps ahead of the recursion, so the eye transfer is
                # done before this slot executes.
                nc.tensor.matmul(out=e_ps_bufs[3][0:1, 0:1],
                                 lhsT=eye_t[0:1, 0:1].bitcast(F32),
                                 rhs=eye_t[0:1, 0:1].bitcast(F32),
                                 start=True, stop=True)
            g_sb = make_G(tau, lt_sb, fresh,
                          apply_lnz=tau % NORM_EVERY == 4 and state["lnz"] is not None)
            ag = nxt("ag", ag_bufs)[:]
            # absorber: carries the A-half alpha wait (so multA carries only
            # the ACT wait; multB carries the B-half alpha wait itself)
            i_d1 = nc.vector.tensor_copy(
                out=dve_scr[0:1, tau % 8:tau % 8 + 1], in_=alpha[0:1, 0:1])
            i_mA = nc.vector.tensor_tensor(
                out=_ap(ag, 0, [[128, 2], [64, 2], [1, 64]]),
                in0=_ap(g_sb, 0, [[128, 2], [64, 2], [1, 64]]),
                in1=_ap(alpha, 0, [[64, 2], [0, 2], [1, 64]]), op=mult)
            add_dep_helper(i_mA.ins, i_d1.ins, False, "dve-order")
            nc.vector.tensor_tensor(
                out=_ap(ag, 256, [[128, 2], [64, 2], [1, 64]]),
                in0=_ap(g_sb, 256, [[128, 2], [64, 2], [1, 64]]),
                in1=_ap(alpha, 128, [[64, 2], [0, 2], [1, 64]]), op=mult)
            # pairsum -> alpha' (prenorm); store to ahist when in output range
            # A-half on DVE; B-half on the (forward-idle) GPSIMD — its result
            # lands just in time for the next step's B-mult
            if L <= tau < L + S:
                dst = ahist[:, (tau - L) * 256:(tau - L + 1) * 256]
            else:
                dst = nxt("aw", aw_bufs)[:]
            nc.vector.tensor_tensor(
                out=_ap(dst, 0, [[64, 2], [32, 2], [1, 32]]),
                in0=_ap(ag, 0, [[128, 2], [64, 2], [2, 32]]),
                in1=_ap(ag, 1, [[128, 2], [64, 2], [2, 32]]), op=add)
            nc.gpsimd.tensor_tensor(
                out=_ap(dst, 128, [[64, 2], [32, 2], [1, 32]]),
                in0=_ap(ag, 256, [[128, 2], [64, 2], [2, 32]]),
                in1=_ap(ag, 257, [[128, 2], [64, 2], [2, 32]]), op=add)
            if zpend is not None:
                # reads a step-old buffer: executes immediately after this
                # step's pairsum, filling the recursion's sem-latency gap
                emit_zred(zpend)
                zpend = None
            alpha = dst
            if tau % NORM_EVERY == NORM_EVERY - 1 and tau <= TL - 4:
                zpend = alpha

        # ---------------- backward ----------------
        # boundary absorber: an explicit DVE wait on the forward pass's last
        # Pool pairsum so later WAW/WAR deps against Pool writes are implied
        nc.vector.tensor_copy(out=dve_scr[0:1, 7:8], in_=alpha[0:1, 128:129])
        beta = nxt("aw", aw_bufs)[:]
        nc.vector.memset(beta, 1.0 / 64)
        zpend = None
        for tau in range(TL - 1, -1, -1):
            # the bwd's first window is the fwd's last -- still resident in
            # SBUF, skip the reload (and its +900ns completion sem at the
            # pass boundary)
            fresh = tau % W == W - 1 and tau != TL - 1
            if fresh:
                lt_sb = load_window(tau // W)
            state["dve_stable"] = (dve_scr[0:1, (tau + 2) % 8:(tau + 2) % 8 + 1]
                                   if tau <= TL - 3 else beta)
            g_sb = make_G(tau, lt_sb, fresh, deint=True,
                          apply_lnz=tau % NORM_EVERY == 3 and state["lnz"] is not None)
            bg = nxt("ag", ag_bufs)[:]
            i_d1 = nc.vector.tensor_copy(
                out=dve_scr[0:1, tau % 8:tau % 8 + 1], in_=beta[0:1, 0:1])
            # layout (f, b, m, k): all operands unit-stride innermost (2x mode)
            i_mA = nc.vector.tensor_tensor(
                out=_ap(bg, 0, [[128, 2], [64, 2], [32, 2], [1, 32]]),
                in0=_ap(g_sb, 0, [[128, 2], [64, 2], [32, 2], [1, 32]]),
                in1=_ap(beta, 0, [[64, 2], [32, 2], [0, 2], [1, 32]]), op=mult)
            add_dep_helper(i_mA.ins, i_d1.ins, False, "dve-order")
            nc.vector.tensor_tensor(
                out=_ap(bg, 256, [[128, 2], [64, 2], [32, 2], [1, 32]]),
                in0=_ap(g_sb, 256, [[128, 2], [64, 2], [32, 2], [1, 32]]),
                in1=_ap(beta, 128, [[64, 2], [32, 2], [0, 2], [1, 32]]), op=mult)
            # pairsum -> beta; the beta consumed by step tau-1's joint is this
            # step's output: store it as bhist column tau-1-L for the
            # deferred joint computation
            if L + 1 <= tau <= L + S:
                dst = bhist[:, (tau - 1 - L) * 256:(tau - L) * 256]
            else:
                dst = nxt("aw", aw_bufs)[:]
            # iterate (f, m, k): out idx 64f + 2k + m ; bg idx 128f + b64 + 32m + k
            nc.vector.tensor_tensor(
                out=_ap(dst, 0, [[64, 2], [1, 2], [2, 32]]),
                in0=_ap(bg, 0, [[128, 2], [32, 2], [1, 32]]),
                in1=_ap(bg, 64, [[128, 2], [32, 2], [1, 32]]), op=add)
            # B-half: the first 10 bwd steps write only aw scratch (no bhist)
            # and run before any jm slab exists -> the Pool is idle there and
            # can take the pairsum exactly like the forward pass
            pb_eng = nc.gpsimd if tau >= TL - 10 else nc.vector
            pb_eng.tensor_tensor(
                out=_ap(dst, 128, [[64, 2], [1, 2], [2, 32]]),
                in0=_ap(bg, 256, [[128, 2], [32, 2], [1, 32]]),
                in1=_ap(bg, 320, [[128, 2], [32, 2], [1, 32]]), op=add)
            if zpend is not None:
                emit_zred(zpend)
                zpend = None
            beta = dst
            if tau % NORM_EVERY == 0 and tau >= 3:
                zpend = beta
            # stream the joint computation on the (otherwise idle) GPSIMD:
            # bhist columns > tau-1-L are complete AND no longer live (the
            # current beta is column tau-1-L) -> Pool does jm = ahist*bhist
            # and the first tree stage for a 16-column slab, overlapping the
            # recursion
            k2 = tau - L
            if 0 <= k2 < S and k2 % 16 == 0:
                base = k2 * 256
                sl = slice(k2 * 256, (k2 + 16) * 256)
                nc.gpsimd.tensor_tensor(out=bhist[:, sl], in0=bhist[:, sl],
                                        in1=ahist[:, sl], op=mult)
                dims = [[256, 16], [32, 8], [1, 16]]
                nc.gpsimd.tensor_tensor(
                    out=_ap(bhist, base, dims), in0=_ap(bhist, base, dims),
                    in1=_ap(bhist, base + 16, dims), op=add)

        # ---------------- deferred joint: remaining tree stages (DVE) -----
        # two column-halves so the Ln/sub/DMA of half 0 overlap half 1's tree
        H = S // 2
        lg_t = ctx.enter_context(nc.sbuf_tensor("lgbuf", [128, S * 8], F32))
        lg = lg_t[:]
        llr_t2 = ctx.enter_context(nc.sbuf_tensor("llrsb", [128, 512], F32))
        llr_sb = llr_t2

        def tree_half(h, prev_fin=None):
            cb, jb = h * H * 256, h * H * 8
            first = None
            for w in (8, 4, 2):
                dims = [[256, H], [32, 8], [1, w]]
                i_t = nc.vector.tensor_tensor(
                    out=_ap(bhist, cb, dims), in0=_ap(bhist, cb, dims),
                    in1=_ap(bhist, cb + w, dims), op=add)
                if first is None:
                    first = i_t
            i_fin = nc.vector.tensor_tensor(
                out=_ap(jsum, jb, [[8, H], [1, 8], [1, 1]]),
                in0=_ap(bhist, cb, [[256, H], [32, 8], [1, 1]]),
                in1=_ap(bhist, cb + 1, [[256, H], [32, 8], [1, 1]]), op=add)
            nc.scalar.activation(out=lg[:, jb:jb + H * 8],
                                 in_=jsum[:, jb:jb + H * 8],
                                 func=mybir.ActivationFunctionType.Ln)
            if prev_fin is not None:
                # keep the scheduler from interleaving the halves stage-major
                # (half 0's Ln must overlap half 1's tree)
                add_dep_helper(first.ins, prev_fin.ins, False, "half-order")
            return i_fin

        def sub_half(h):
            jb = h * H * 8
            nc.vector.tensor_tensor(
                out=_ap(llr_sb[:], h * H, [[128, CF], [1, H]]),
                in0=_ap(lg, jb, [[2, CF], [8, H]]),
                in1=_ap(lg, jb + 1, [[2, CF], [8, H]]),
                op=mybir.AluOpType.subtract)

        # half 1 first: its Pool jm-slabs (high columns) finish early in the
        # backward pass, while half 0's last slab lands only at the very end
        def out_dma(h):
            src = _ap(llr_sb[:], h * H, [[128, CF], [1, H]])
            dstp = bass.AP(tensor=out_d[:].tensor, offset=h * H,
                           ap=[[2048, 32], [512, 4], [128, 4], [1, H]])
            nc.sync.dma_start(out=dstp, in_=src)

        fin1 = tree_half(1)
        tree_half(0, prev_fin=fin1)
        sub_half(1)
        sub_half(0)
        # single output DMA: split halves land on different HW queues (8-way
        # round-robin) and the final drain can carry only one wait
        src_ap = llr_sb[:].rearrange("p (f k) -> p f k", f=4)
        dst_ap = bass.AP(tensor=out_d[:].tensor, offset=0,
                         ap=[[2048, 32], [512, 4], [128, 4], [1, 128]])
        nc.sync.dma_start(out=dst_ap, in_=src_ap)

        if dbg:
            nc.sync.dma_start(out=dbg_jsum[:], in_=jsum)
            nc.sync.dma_start(out=dbg_ahist[:, 0:256], in_=ahist[:, 0:256])
            nc.sync.dma_start(out=dbg_ahist[:, 256:512], in_=ahist[:, 127 * 256:128 * 256])
            nc.sync.dma_start(out=dbg_g[:], in_=g_bufs[0][:])
    return nc


_ENG_SELF = {"PE": "PE_", "DVE": "DVE_", "Activation": "Activation_",
             "Pool": "Pool_", "SP": "SP_"}


def _prune_waits(nc):
    """Drop sem waits already implied, so each instruction carries <=1.

    HW structs accept one sync wait per instruction. Tile emits waits that
    are provably satisfied at issue. Vector-clock rules:
      - cross-engine sems: knowledge from transitive joins of kept waits
      - self sems (same engine): only monotone vs explicitly-waited values
        (ACT/DVE completion is not implied by issue order); PE and DMA
        queues complete in order, so own-increment knowledge counts there.
    """
    know = {}        # proc -> {sem_id: known completed value}
    safe = {}        # proc -> {sem_id: completion-proven value (waits/joins
                     #          only, no own-increment issue-order knowledge)}
    waited_max = {}  # proc -> {sem_id: max explicitly waited}
    sem_total = {}   # sem_id -> running total
    hist = {}        # sem_id -> [(total_after, snapshot)]
    out_dma_sems = set()
    bad = []
    for b in nc.m.functions[0].blocks:
        for i in b.instructions:
            si = i.sync_info
            op = str(getattr(i, "opcode", type(i).__name__))
            if si is None:
                continue
            upds = [u for u in (si.on_update or [])
                    if u.sync_type == "semaphore"
                    and u.update_mode in ("sem-inc", "sem-add-imm")]
            if "DMACopy" in op and upds:
                proc = str(upds[0].ant_name)
                outs = getattr(i, "outs", None) or []
                if outs and "llr_out" in str(getattr(outs[0], "memref", "")):
                    out_dma_sems.add(upds[0].id)
            else:
                proc = getattr(i.engine, "value", str(i.engine))
            k = know.setdefault(proc, {})
            ks = safe.setdefault(proc, {})
            wm = waited_max.setdefault(proc, {})
            in_order = proc == "PE" or proc.startswith("DMAHW")
            if "Drain" in op and si.on_wait and len(si.on_wait) > 1:
                best = {}
                for w in si.on_wait:
                    if w.id in out_dma_sems and (
                            w.id not in best
                            or (w.wait_value or 0) > (best[w.id].wait_value or 0)):
                        best[w.id] = w
                si.on_wait = list(best.values())
                continue
            skip = ("Drain" in op) or ("EventSem" in op)
            ow = list(si.on_wait or [])
            if ow and not skip:
                # evaluate cross-engine waits first: their hist joins can
                # prove completion of this engine's own earlier instructions,
                # letting the self-wait checks below prune
                def _selfish(w):
                    nm = str(w.ant_name)
                    return nm == proc or nm.startswith(proc + "_")
                ow.sort(key=_selfish)
                keep = []
                for w in ow:
                    if (w.sync_type != "semaphore"
                            or w.wait_mode != "sem-ge-imm"
                            or w.wait_value is None
                            or str(w.ant_name).startswith("barrier")):
                        keep.append(w)
                        continue
                    v = w.wait_value
                    nm = str(w.ant_name)
                    is_self = nm == proc or nm.startswith(proc + "_")
                    if is_self:
                        implied = (wm.get(w.id, -1) >= v
                                   or ks.get(w.id, 0) >= v
                                   or (in_order and k.get(w.id, 0) >= v))
                    else:
                        implied = (k.get(w.id, 0) >= v
                                   or wm.get(w.id, -1) >= v)
                    if implied:
                        continue
                    keep.append(w)
                    wm[w.id] = max(wm.get(w.id, -1), v)
                    for tot, snap in hist.get(w.id, ()):
                        if tot >= v:
                            for s2, v2 in snap.items():
                                if k.get(s2, 0) < v2:
                                    k[s2] = v2
                                if ks.get(s2, 0) < v2:
                                    ks[s2] = v2
                            break
                    if k.get(w.id, 0) < v:
                        k[w.id] = v
                    if ks.get(w.id, 0) < v:
                        ks[w.id] = v
                if len(keep) != len(ow):
                    si.on_wait = keep
                    ow = keep
                if len(ow) > 1:
                    bad.append((i.name, op,
                                [(x.ant_name, x.wait_value) for x in ow]))
            for u in upds:
                tot = sem_total.get(u.id, 0) + (u.update_value or 0)
                sem_total[u.id] = tot
                k[u.id] = tot
                hist.setdefault(u.id, []).append((tot, dict(k)))
    if bad:
        raise RuntimeError(f"{len(bad)} insts still multi-wait: {bad[:8]}")
    return nc


def _get_nc():
    if "nc" not in _NC_CACHE:
        _NC_CACHE["nc"] = _prune_waits(build_nc())
    return _NC_CACHE["nc"]


# ---------------- host-side layout ----------------
def _prep_core(llr_ch_c, llr_a_c):
    """llr_ch_c [32, 4096], llr_a_c [32, 2048] -> llr_t [16, TL*128] f32."""
    lc = np.zeros((B_CORE, T + 2 * L, 2), np.float32)
    lc[:, L:L + T] = llr_ch_c.reshape(B_CORE, T, 2)
    la = np.full((B_CORE, T + 2 * L), PAD_A, np.float32)
    la[:, L:L + T] = llr_a_c
    # pad-step exponent shift: cancels the e^{+8}/step growth of the la=+16
    # pilots so normalization can lag a few steps without bf16 overflow
    sh1 = np.zeros(T + 2 * L, np.float32)
    sh1[:L] = -PAD_A
    sh1[T + L:] = -PAD_A
    sh = np.broadcast_to(sh1, (B_CORE, T + 2 * L))
    # windows [B, C, TL, comp]
    idx = (np.arange(C)[:, None] * S + np.arange(TL)[None, :])  # [C, TL]
    w = np.stack([la[:, idx], lc[:, idx, 0], lc[:, idx, 1], sh[:, idx]], -1)
    # chunk c = g*4+f ; row = cw*4+g ; llr_t[f*4+comp, tau*128+row]
    w = w.reshape(B_CORE, 4, 4, TL, NCOMP)        # [cw, g, f, tau, comp]
    w = w.transpose(2, 4, 3, 0, 1)                # [f, comp, tau, cw, g]
    return np.ascontiguousarray(w.reshape(4 * NCOMP, TL * 128))


def _run(llr_ch, llr_a, trace=False):
    nc = _get_nc()
    in_maps = []
    for core in range(N_CORES):
        sl = slice(core * B_CORE, (core + 1) * B_CORE)
        in_maps.append({
            "llr_t": _prep_core(np.asarray(llr_ch[sl], np.float32),
                                np.asarray(llr_a[sl], np.float32)),
            "sign": SIGN_BD,
            "eye": EYE_M2,
        })
    res = run_bass_kernel_spmd(nc, in_maps, core_ids=list(range(N_CORES)),
                               trace=trace)
    out = np.concatenate([r["llr_out"] for r in res.results], 0)
    return out.astype(np.float32), res


def kernel(llr_ch, llr_a):
    out, _ = _run(llr_ch, llr_a, trace=False)
    return out

